# revision 16
# baseline (speedup 1.0000x reference)
"""Deformable-separate-conv2d Trainium2 kernel (8-core data-parallel).

Per core = one (batch, 48-row H-slice) shard:
  - Small 3x3 convs (offsets / modulation masks) on PE as shifted matmuls.
  - Bilinear sampling as a dense 3x3 shift window with per-pixel hat
    weights hat(t)=relu(1-|t|):
        sampled[g,c,k,h,w] = sum_{ry,rx in -1..1}
            hat(dy-ry)*hat(dx-rx) * x[g,c, h+ky+ry, w+kx+rx]
    exact whenever -1 <= dy,dx < 1.  Rare outliers (semantic branch tail)
    are patched exactly on the host from the device-computed offsets.
  - Layout: W on partitions (96 wide), (chunk, c, h) on free dim, h
    innermost (packed bf16 -> DVE 4x).  W-shifts come from 5 HBM-loaded
    shifted slab copies; h-shifts are free-dim offsets.
  - Modulated samples round-trip through HBM to become channel-major, then
    one PE matmul contracts (g,c,k) (+1 mask row per (g,k) that yields the
    update-mask row 64).
"""

import os
import sys
import numpy as np

sys.path.insert(0, "/opt/trn_rl_repo")

B, CIN, COUT, H, W = 2, 64, 64, 192, 192
G, KK, CG = 2, 9, 32
NCORE = 8
NSLICE = 4
OH = H // NSLICE      # 48
SH = OH + 6           # 54
WC = 96
NCH = 2
CC = 33               # 32 data channels + sampled-mask channel
NGRP = 6
GR = 3 * CC           # 99
NPX = OH * W          # 9216
WBLK = 8
PXC = WBLK * OH       # 384

_compiled = None
_last_res = None


def _split_waits(nc, mybir, limit=1):
    """Walrus encodes only a small number of sync-wait commands per
    instruction; hoist extra waits into standalone EventSemaphore ops."""
    n_split = 0
    for func in nc.m.functions:
        for blk in func.blocks:
            new_insts = []
            changed = False
            for ins in blk.instructions:
                si = ins.sync_info
                waits = list(si.on_wait) if si is not None and si.on_wait else []
                if len(waits) > limit and ins.opcode != "EventSemaphore":
                    keep = waits[-limit:]
                    for w in waits[:-limit]:
                        ev = mybir.InstNoOp(
                            name=f"wsplit_{nc.next_id()}")
                        ev.engine = ins.engine
                        ev.sync_info = mybir.SyncInfo(on_wait=[w], on_update=[])
                        new_insts.append(ev)
                        n_split += 1
                    ins.sync_info = mybir.SyncInfo(
                        on_wait=keep,
                        on_update=list(si.on_update) if si.on_update else [])
                    changed = True
                new_insts.append(ins)
            if changed:
                blk.instructions = new_insts
    return n_split


def _build_bass(split=True):
    import concourse.bass as bass
    import concourse.mybir as mybir
    import concourse.tile as tile
    from concourse import tile_sem_assignment, tile_scheduler
    # Cap distinct DMA-completion semaphore lanes so no instruction
    # accumulates more sync-waits than the ISA wait-field limit.
    tile_sem_assignment.NUM_HWDGE_SEMS = 3
    tile_scheduler.NUM_HWDGE_SEMS = 3
    from concourse.masks import make_identity

    f32 = mybir.dt.float32
    bf16 = mybir.dt.bfloat16
    A = mybir.AluOpType
    AF = mybir.ActivationFunctionType

    nc = bass.Bass()

    XWH = nc.declare_dram_parameter("xwh", [108, NCH, G, CC, SH], bf16, isOutput=False)
    XC2 = nc.declare_dram_parameter("xc2", [128, 51, 194], f32, isOutput=False)
    MC9 = nc.declare_dram_parameter("mc9", [9, OH, 192], f32, isOutput=False)
    LWP = nc.declare_dram_parameter("lwp", [3, 128, 27], f32, isOutput=False)
    LW1 = nc.declare_dram_parameter("lw1", [3, 64, 27], f32, isOutput=False)
    MW9 = nc.declare_dram_parameter("mw9", [9, 27], f32, isOutput=False)
    WRG = nc.declare_dram_parameter("wrg", [NGRP, GR, 65], bf16, isOutput=False)
    BIA = nc.declare_dram_parameter("bia", [65, 1], f32, isOutput=False)
    RC3 = nc.declare_dram_parameter("rc3", [96, 3], f32, isOutput=False)
    ON1 = nc.declare_dram_parameter("on1", [1, 64], bf16, isOutput=False)
    OUTD = nc.declare_dram_parameter("outd", [65, NPX], f32, isOutput=True)
    OFFO = nc.declare_dram_parameter("offo", [59, NPX], f32, isOutput=True)

    SMH = nc.dram_tensor("smh", [NGRP, WC, NCH, GR, OH], bf16)

    with tile.TileContext(nc) as tc:
        with (
            tc.tile_pool(name="const", bufs=1) as constp,
            tc.tile_pool(name="wpool", bufs=1) as wpool,
        ):
            ident = constp.tile([59, 59], f32)
            make_identity(nc, ident[:])
            rcon = constp.tile([96, 3], f32)
            nc.sync.dma_start(out=rcon[:], in_=RC3[:])
            bias_t = constp.tile([65, 1], f32)
            nc.sync.dma_start(out=bias_t[:], in_=BIA[:])
            ones_t = constp.tile([1, 64], bf16)
            nc.sync.dma_start(out=ones_t[:], in_=ON1[:])

            w2 = wpool.tile([96, NCH, G, KK, 3, 3, OH], bf16)
            dmk = wpool.tile([96, NCH, G, KK, OH], bf16)

            # ---------------- P1: convs ----------------
            offc_cm = tc.tile_pool(name="offc", bufs=1)
            offcp = offc_cm.__enter__()
            offc = offcp.tile([59, NPX], f32)
            nc.gpsimd.memset(offc[:], 0.0)
            with (
                tc.tile_pool(name="convw", bufs=1) as convwp,
                tc.tile_pool(name="convx", bufs=1) as convxp,
                tc.tile_pool(name="cpsum", bufs=2, space="PSUM") as cpsum,
            ):
                lwp_t = []
                lw1_t = []
                for dx in range(3):
                    t = convwp.tile([128, 27], f32, tag=f"lwp{dx}")
                    nc.sync.dma_start(out=t[:], in_=LWP[dx])
                    lwp_t.append(t)
                    t1 = convwp.tile([64, 27], f32, tag=f"lw1{dx}")
                    nc.sync.dma_start(out=t1[:], in_=LW1[dx])
                    lw1_t.append(t1)
                mw9_t = convwp.tile([9, 27], f32)
                nc.sync.dma_start(out=mw9_t[:], in_=MW9[:])
                xc2_t = convxp.tile([128, 51, 194], f32)
                nc.sync.dma_start(out=xc2_t[:], in_=XC2[:])
                mc9_t = convxp.tile([9, OH, 192], f32)
                nc.sync.dma_start(out=mc9_t[:], in_=MC9[:])

                for ci in range(OH // 2):
                    psA = cpsum.tile([27, 2, 192], f32, tag="psA")
                    psB = cpsum.tile([27, 2, 192], f32, tag="psB")
                    for dx in range(3):
                        nc.tensor.matmul(
                            psA[:],
                            lwp_t[dx][:],
                            xc2_t[:, 2 * ci : 2 * ci + 2, dx : dx + 192],
                            start=(dx == 0),
                            stop=False,
                        )
                    for dx in range(3):
                        nc.tensor.matmul(
                            psA[:],
                            lw1_t[dx][:],
                            xc2_t[0:64, 2 * ci + 2 : 2 * ci + 4, dx : dx + 192],
                            start=False,
                            stop=(dx == 2),
                        )
                    nc.tensor.matmul(
                        psB[:],
                        mw9_t[:],
                        mc9_t[:, 2 * ci : 2 * ci + 2, :],
                        start=True,
                        stop=True,
                    )
                    nc.scalar.activation(
                        offc[0:27, 384 * ci : 384 * (ci + 1)],
                        psA[:].rearrange("p a b -> p (a b)"),
                        AF.Copy,
                    )
                    nc.scalar.activation(
                        offc[32:59, 384 * ci : 384 * (ci + 1)],
                        psB[:].rearrange("p a b -> p (a b)"),
                        AF.Copy,
                    )
            nc.sync.dma_start(out=OFFO[:], in_=offc[:])

            # ---------------- P2+P3: transpose offsets, build weights -------
            if True:
                with (
                    tc.tile_pool(name="offw", bufs=1) as offwp,
                    tc.tile_pool(name="tpsum", bufs=4, space="PSUM") as tpsum,
                    tc.tile_pool(name="hatp", bufs=1) as hatp,
                ):
                    offw = offwp.tile([96, NCH, OH, 59], f32)
                    for ch in range(NCH):
                        for r in range(OH):
                            tp = tpsum.tile([96, 59], f32, tag="tp")
                            nc.tensor.transpose(
                                tp[:],
                                offc[:, 192 * r + 96 * ch : 192 * r + 96 * ch + 96],
                                ident[:],
                            )
                            nc.scalar.activation(offw[:, ch, r], tp[:], AF.Copy)

                    wy3 = hatp.tile([96, NCH, G, KK, 3, OH], f32, tag="wy3")
                    wx3 = hatp.tile([96, NCH, G, KK, 3, OH], f32, tag="wx3")
                    # ISA limit: <=3 free dims per DVE operand -> loop ch.
                    for g in range(G):
                        for ch in range(NCH):
                            for wt, ax in ((wy3, 0), (wx3, 1)):
                                cbase = (32 if g == 0 else 0) + ax
                                dv = offw[:, ch, :, cbase : cbase + 18 : 2].transpose(
                                    (0, 2, 1)
                                )  # [96, 9k, 48h]
                                dst = wt[:, ch, g]  # [96, 9, 3, 48]
                                dvb = dv.unsqueeze(2).broadcast_to((96, KK, 3, OH))
                                rcb = (
                                    rcon[:]
                                    .unsqueeze(1)
                                    .unsqueeze(3)
                                    .broadcast_to((96, KK, 3, OH))
                                )
                                nc.vector.tensor_tensor(dst, dvb, rcb, A.subtract)
                                nc.vector.scalar_tensor_tensor(
                                    dst, dst, -1.0, dst, A.mult, A.max
                                )
                                nc.vector.tensor_scalar(
                                    dst, dst, -1.0, 1.0, A.mult, A.add
                                )
                                nc.vector.tensor_scalar_max(dst, dst, 0.0)
                            cbase = 50 if g == 0 else 18
                            dv = offw[:, ch, :, cbase : cbase + 9].transpose((0, 2, 1))
                            nc.scalar.activation(dmk[:, ch, g], dv, AF.Sigmoid)

                    for ch in range(NCH):
                        for g in range(G):
                            for ry in range(3):
                                nc.vector.tensor_tensor(
                                    w2[:, ch, g, :, ry],
                                    wy3[:, ch, g, :, ry]
                                    .unsqueeze(2)
                                    .broadcast_to((96, KK, 3, OH)),
                                    wx3[:, ch, g],
                                    A.mult,
                                )

                offc_cm.__exit__(None, None, None)

                # ---------------- P4: dense 3x3 window MACs ----------------
                with (
                    tc.tile_pool(name="xws", bufs=1) as xwsp,
                    tc.tile_pool(name="smp", bufs=2) as smp,
                ):
                    xws = {}
                    for s in range(-2, 3):
                        for g in range(G):
                            t = xwsp.tile([96, NCH, CC, SH], bf16, tag=f"xw{s}_{g}")
                            for ch in range(NCH):
                                nc.sync.dma_start(
                                    out=t[:, ch], in_=XWH[6 + s : 102 + s, ch, g]
                                )
                            xws[(s, g)] = t

                    for g in range(G):
                        for k in range(KK):
                            ky, kx = k // 3 - 1, k % 3 - 1
                            sm = smp.tile([96, NCH, CC, OH], bf16, tag="sm")
                            tmp = smp.tile([96, NCH, CC, OH], bf16, tag="smt")
                            first = True
                            for ry in (-1, 0, 1):
                                for rx in (-1, 0, 1):
                                    x_in = xws[(kx + rx, g)][
                                        :, :, :, 3 + ky + ry : 3 + ky + ry + OH
                                    ]
                                    wv = (
                                        w2[:, :, g, k, ry + 1, rx + 1]
                                        .unsqueeze(2)
                                        .broadcast_to((96, NCH, CC, OH))
                                    )
                                    if first:
                                        nc.vector.tensor_tensor(
                                            sm[:], x_in, wv, A.mult
                                        )
                                        first = False
                                    else:
                                        nc.vector.tensor_tensor(
                                            tmp[:], x_in, wv, A.mult
                                        )
                                        nc.vector.tensor_tensor(
                                            sm[:], sm[:], tmp[:], A.add
                                        )
                            dv = (
                                dmk[:, :, g, k]
                                .unsqueeze(2)
                                .broadcast_to((96, NCH, CG, OH))
                            )
                            nc.vector.tensor_tensor(
                                sm[:, :, 0:CG, :], sm[:, :, 0:CG, :], dv, A.mult
                            )
                            gk = g * KK + k
                            grp, slot = gk // 3, gk % 3
                            nc.sync.dma_start(
                                out=SMH[grp, :, :, CC * slot : CC * slot + CC, :],
                                in_=sm[:],
                            )

            # ---------------- P5: einsum ----------------
            with (
                tc.tile_pool(name="wrgp", bufs=1) as wrgp,
                tc.tile_pool(name="rhs", bufs=4) as rhsp,
                tc.tile_pool(name="outs", bufs=1) as outsp,
                tc.tile_pool(name="epsum", bufs=4, space="PSUM") as epsum,
            ):
                wrg_t = wrgp.tile([GR, NGRP, 65], bf16)
                nc.sync.dma_start(out=wrg_t[:], in_=WRG[:].transpose((1, 0, 2)))
                outs = outsp.tile([65, NPX], f32)
                umb = outsp.tile([64, NPX], f32)

                for ch in range(NCH):
                    for wb in range(WC // WBLK):
                        ps = epsum.tile([65, PXC], f32, tag="eps")
                        rhts = []
                        for grp in range(NGRP):
                            rh = rhsp.tile(
                                [GR, WBLK, OH], bf16, tag=f"rh{grp % 2}"
                            )
                            nc.sync.dma_start(
                                out=rh[:],
                                in_=SMH[
                                    grp, WBLK * wb : WBLK * (wb + 1), ch
                                ].transpose((1, 0, 2)),
                            )
                            rhts.append(rh)
                        for grp in range(NGRP):
                            nc.tensor.matmul(
                                ps[:],
                                wrg_t[:, grp],
                                rhts[grp][:].rearrange("p a b -> p (a b)"),
                                start=(grp == 0),
                                stop=(grp == NGRP - 1),
                            )
                        col = 4608 * ch + PXC * wb
                        nc.vector.tensor_scalar(
                            outs[:, col : col + PXC],
                            ps[:],
                            bias_t[:],
                            None,
                            A.add,
                        )

                nc.scalar.activation(
                    outs[64:65, :], outs[64:65, :], AF.Relu, scale=float(2 * CG)
                )
                nc.vector.tensor_scalar_min(outs[64:65, :], outs[64:65, :], 1.0)
                umr16 = outsp.tile([1, NPX], bf16)
                nc.vector.tensor_copy(umr16[:], outs[64:65, :])
                for pc in range(NPX // 512):
                    bps = epsum.tile([64, 512], f32, tag="bps")
                    nc.tensor.matmul(
                        bps[:],
                        ones_t[:],
                        umr16[:, 512 * pc : 512 * (pc + 1)],
                        start=True,
                        stop=True,
                    )
                    nc.scalar.activation(
                        umb[:, 512 * pc : 512 * (pc + 1)], bps[:], AF.Copy
                    )
                nc.vector.tensor_tensor(
                    outs[0:64, :], outs[0:64, :], umb[:], A.mult
                )
                nc.sync.dma_start(out=OUTD[:], in_=outs[:])

    if split:
        _split_waits(nc, mybir)
    return nc


def _host_prep(input_f, mask_f, weight, bias):
    import ml_dtypes
    f32 = np.float32
    bf16 = ml_dtypes.bfloat16

    # offsets/masks conv weights are bound at build time? no - they are
    # inputs; build lhsT layouts here.
    return f32, bf16


def _hat_inputs(inputs):
    pass


def _make_in_maps(input_f, mask_f, weight, bias, sem_w, reg_w, m1_w, m2_w):
    import ml_dtypes
    f32 = np.float32
    bf16 = ml_dtypes.bfloat16

    w_in = np.concatenate([sem_w, m2_w], axis=0)  # [27, 64, 3, 3]
    LWP = np.zeros((3, 128, 27), f32)
    LW1 = np.zeros((3, 64, 27), f32)
    for dx in range(3):
        LWP[dx, 0:64] = w_in[:, :, 0, dx].T
        LWP[dx, 64:128] = w_in[:, :, 1, dx].T
        LW1[dx] = w_in[:, :, 2, dx].T
    w_mk = np.concatenate([reg_w, m1_w], axis=0)[:, 0]  # [27, 3, 3]
    MW9 = np.zeros((9, 27), f32)
    for t in range(9):
        MW9[t] = w_mk[:, t // 3, t % 3]

    wr = weight.reshape(COUT, G, CG, KK)
    WRG = np.zeros((NGRP, GR, 65), f32)
    for g in range(G):
        for k in range(KK):
            gk = g * KK + k
            grp, slot = gk // 3, gk % 3
            WRG[grp, CC * slot : CC * slot + CG, 0:64] = wr[:, g, :, k].T
            WRG[grp, CC * slot + CG, 64] = 1.0
    BIA = np.zeros((65, 1), f32)
    BIA[0:64, 0] = bias
    RC3 = np.broadcast_to(np.array([-1.0, 0.0, 1.0], f32), (96, 3)).copy()

    ON1 = np.ones((1, 64), f32).astype(bf16)
    shared = dict(lwp=LWP, lw1=LW1, mw9=MW9, wrg=WRG.astype(bf16), bia=BIA,
                  rc3=RC3, on1=ON1)

    xg_all = input_f.reshape(B, G, CG, H, W)
    in_maps = []
    for core in range(NCORE):
        b, q = core // NSLICE, core % NSLICE
        h0 = q * OH

        # XWH [108, ch, g, 33, 54]; wi -> w = ch*96 + wi - 6; h rows h0-3..h0+50
        xwh = np.zeros((108, NCH, G, CC, SH), f32)
        r0 = h0 - 3
        rs0, rs1 = max(r0, 0), min(h0 + 51, H)
        for ch in range(NCH):
            w_lo = ch * WC - 6
            # valid wi range for this chunk
            wi0 = max(0, -w_lo)
            wi1 = min(108, W - w_lo)
            src = xg_all[b, :, :, rs0:rs1, w_lo + wi0 : w_lo + wi1]
            # src [G, CG, rows, wn] -> xwh[wi, ch, G, c, rows]
            xwh[wi0:wi1, ch, :, 0:CG, rs0 - r0 : rs1 - r0] = src.transpose(
                3, 0, 1, 2
            )
            msrc = mask_f[b, 0, rs0:rs1, w_lo + wi0 : w_lo + wi1]
            xwh[wi0:wi1, ch, :, CG, rs0 - r0 : rs1 - r0] = msrc.T[:, None, :]

        xc2 = np.zeros((128, 51, 194), f32)
        for j in range(51):
            r = h0 - 1 + j
            if 0 <= r < H:
                xc2[0:64, j, 1:193] = input_f[b, :, r, :]
            if 0 <= r + 1 < H:
                xc2[64:128, j, 1:193] = input_f[b, :, r + 1, :]

        mc9 = np.zeros((9, OH, 192), f32)
        mp = np.pad(mask_f[b, 0], 1)
        for t in range(9):
            ty, tx = t // 3, t % 3
            mc9[t] = mp[h0 + ty : h0 + ty + OH, tx : tx + 192]

        m = dict(shared)
        m.update(xwh=xwh.astype(bf16), xc2=xc2, mc9=mc9)
        in_maps.append(m)
    return in_maps


def _host_patch(out_full, um_full, offo_cores, input_f, mask_f, weight, bias):
    """Exact fp32 recompute of pixels whose offsets fall outside [-1, 1)."""
    wr = weight.reshape(COUT, G, CG, KK)
    xg = input_f.reshape(B, G, CG, H, W)
    for core in range(NCORE):
        b, q = core // NSLICE, core % NSLICE
        h0 = q * OH
        offc = offo_cores[core].reshape(59, OH, 192)
        off = np.zeros((G, KK, 2, OH, 192), np.float32)
        off[1, :, 0] = offc[0:18:2]
        off[1, :, 1] = offc[1:18:2]
        off[0, :, 0] = offc[32:50:2]
        off[0, :, 1] = offc[33:51:2]
        dm = np.zeros((G, KK, OH, 192), np.float32)
        dm[1] = 1.0 / (1.0 + np.exp(-offc[18:27]))
        dm[0] = 1.0 / (1.0 + np.exp(-offc[50:59]))

        bad = np.any((off >= 1.0) | (off < -1.0), axis=(0, 1, 2))
        ys, xs = np.nonzero(bad)
        for hh, ww in zip(ys, xs):
            h = h0 + hh
            acc = np.zeros(COUT, np.float64)
            ssum = 0.0
            for g in range(G):
                for k in range(KK):
                    ky, kx = k // 3 - 1, k % 3 - 1
                    py = h + ky + off[g, k, 0, hh, ww]
                    px = ww + kx + off[g, k, 1, hh, ww]
                    y0, x0 = int(np.floor(py)), int(np.floor(px))
                    ly, lx = py - y0, px - x0
                    sv = np.zeros(CG, np.float64)
                    sm_mask = 0.0
                    for dy2, wy in ((0, 1 - ly), (1, ly)):
                        for dx2, wx in ((0, 1 - lx), (1, lx)):
                            yy, xx2 = y0 + dy2, x0 + dx2
                            if 0 <= yy < H and 0 <= xx2 < W:
                                sv += wy * wx * xg[b, g, :, yy, xx2]
                                sm_mask += wy * wx * mask_f[b, 0, yy, xx2]
                    acc += wr[:, g, :, k] @ (sv * dm[g, k, hh, ww])
                    ssum += sm_mask
            um = np.clip(2.0 * CG * ssum, 0.0, 1.0)
            out_full[b, :, h, ww] = (acc + bias) * um
            um_full[b, 0, h, ww] = um
    return out_full, um_full


def kernel(**inputs):
    global _compiled
    from concourse.bass_utils import run_bass_kernel_spmd

    input_f = np.asarray(inputs["input"], np.float32)
    mask_f = np.asarray(inputs["mask_in"], np.float32)
    weight = np.asarray(inputs["weight"], np.float32)
    bias = np.asarray(inputs["bias"], np.float32)
    sem_w = np.asarray(inputs["sem_w"], np.float32)
    reg_w = np.asarray(inputs["reg_w"], np.float32)
    m1_w = np.asarray(inputs["m1_w"], np.float32)
    m2_w = np.asarray(inputs["m2_w"], np.float32)
    for k in ("sem_b", "reg_b", "m1_b", "m2_b"):
        assert np.allclose(np.asarray(inputs[k]), 0.0), f"{k} != 0 unsupported"

    if _compiled is None:
        _compiled = _build_bass()
    nc = _compiled

    in_maps = _make_in_maps(input_f, mask_f, weight, bias, sem_w, reg_w,
                            m1_w, m2_w)
    trace = bool(int(os.environ.get("KERNEL_TRACE", "0")))
    res = run_bass_kernel_spmd(nc, in_maps, list(range(NCORE)), trace=trace)
    global _last_res
    _last_res = res

    out_full = np.zeros((B, COUT, H, W), np.float32)
    um_full = np.zeros((B, 1, H, W), np.float32)
    offo_cores = []
    for core in range(NCORE):
        b, q = core // NSLICE, core % NSLICE
        h0 = q * OH
        od = np.asarray(res.results[core]["outd"], np.float32)
        offo_cores.append(np.asarray(res.results[core]["offo"], np.float32))
        blk = od.reshape(65, NCH, WC // WBLK, WBLK, OH)
        img = blk.transpose(0, 4, 1, 2, 3).reshape(65, OH, W)
        out_full[b, :, h0 : h0 + OH] = img[0:64]
        um_full[b, 0, h0 : h0 + OH] = img[64]

    out_full, um_full = _host_patch(out_full, um_full, offo_cores, input_f,
                                    mask_f, weight, bias)
    return out_full, um_full
